# revision 37
# baseline (speedup 1.0000x reference)
"""Trainium2 Bass kernel for the quantized BasicBlock (nn_BasicBlock_15436112462307).

Strategy
--------
Data-parallel over batch: 64 images -> 8 cores x 8 images. Weights/BN replicated.

fake_quant makes every conv operand an exact small integer (-7..7) times a
global fp32 scale.  We factor the scales out on the host and feed pure
integers to the PE as fp8e4 (integers <=7 are exact in fp8e4), using
perf_mode=DoubleRow so one matmul contracts all 256 input channels
(lhsT [128,2,128] / rhs [128,2,N]) at 2x fp8 rate.  PSUM accumulates the
integer dot products exactly in fp32, so the conv itself is EXACT; all
rounding happens only in the per-channel epilogues, which replicate the
reference's fp32 arithmetic.

Spatial layout: each 28x28 image is zero-padded to 30 rows x 29 cols and
flattened; ONE zero column is shared as the right-pad of row h and the
left-pad of row h+1, so every 3x3 conv tap is a pure diagonal shift in the
flat index -> conv = 9 accumulating matmuls over contiguous windows, with
only 1 garbage column per 29 discarded in the epilogue APs.

Epilogue 1 (conv1 -> conv2 input):  q2 = rne(clip(P1*(7*sx*sw1*inv1) + 7*b1, +-7))
using the fp32 magic-number trick (+-1.5*2^23) for round-to-nearest-even;
the result is an exact integer written directly as fp8 into the padded conv2
input buffer.  The activation fake-quant scale alpha2 = max|hardtanh(...)| is
1.0 whenever anything clips (always, for this distribution); the kernel
computes max|.| on device and the host verifies it is exactly 7.0, falling
back to an exact numpy implementation otherwise.

Epilogue 2: y = clip(P2*(s2*sw2*inv2) + (x*inv2 + b2), +-1); the residual
affine x*inv2+b2 is precomputed on the host, so the device does one fused
scalar_tensor_tensor (scale+add) plus the clip on VectorE.

Inputs stream in staged DMAs in strict first-use order on one HWDGE
queue: a minimal gating chunk (conv1/cot0 taps0-2 + vec + img0, 2576B/
partition) with taps3-8 and [img1 + conv1/cot1 weights] ordering-chained
right behind it (same-queue FIFO drain, no receipt serialization), then
the bulk chunks receipt-chained so the 8 cores' aggregate HBM load stays
staggered.  64 small junk matmuls (N=64, ~53ns each) on zeroed SBUF fill
the first-chunk DMA wait, so the PE HAM clock-gate is warm (2.4GHz) and
the queue is drained right when the gating receipt lands (~11us in); the
first conv1 phase is image-major so it gates only on img0's receipt.
Measured ~116.4us: preamble (SPMD entry barriers + IO-table loads,
~6.8us, +-0.4 run-to-run) + gating-chunk DMA receipt (~4.0us, junk-
covered) + the 576-matmul stream at the DoubleRow floor (~99.7us, zero
gaps, 406 cycles + ~5ns each) + tail (last epilogue chain + y-DMA HBM
write receipt + exit barriers, ~5.5us).
Measured dead ends: 4D windowed rhs APs (392 valid cols) stream no
faster (AP dim-crossing burns the saved cycles) and add NX decode cost;
Winograd F(2,3) would cut PE time 16us but its 4 extra PSUM-source DVE
combines per phase exceed the DVE budget; finer DMA chunking delays the
critical receipt (SDMA round-robins rings/queues at packet granularity).
"""

import numpy as np
import ml_dtypes

EPS = np.float32(1e-5)
NCORES = 8
B, C, H, W = 64, 256, 28, 28
BC = B // NCORES            # images per core
IMS = 880                   # padded (30 rows x 29 cols = 870) image stride;
                            # one zero col shared as right-pad of row h and
                            # left-pad of row h+1
NT = 406                    # matmul N: 14 padded rows x 29
MAGIC = np.float32(12582912.0)  # 1.5 * 2^23
F8NP = ml_dtypes.float8_e4m3

WB = 36 * 2 * 128                   # 9216 bytes/partition of int weights
WG = 2 * 128                        # one weight group (tap)
VB = 48                             # 40B of fp32 epilogue vecs + 8B pad
IMB = 2 * IMS                       # one image (both channel halves)
X1B = BC * IMB                      # 14592 bytes/partition of int inputs
# staged layout: [w(conv1,cot0,taps0-2) | vec | img0 | taps3-8 | img1 |
#                 w rest | imgs2-7] -- strict first-use order so the gating
# first chunk is minimal and the rest FIFO-drains behind it
W0B = 9 * WG
VOFF = 3 * WG
X0OFF = VOFF + VB
W0bOFF = X0OFF + IMB                # conv1 cot0 taps 3-8
X1OFF = W0bOFF + 6 * WG             # img1
WROFF = X1OFF + IMB
X2OFF = WROFF + (WB - W0B)
INPB = X2OFF + 6 * IMB

_BUILT = None  # cached (nc,) so repeat calls skip IR building + compile


# ----------------------------------------------------------------- host math
def _quant_int(v):
    """Exact replica of the reference fake_quant grid; returns integer part."""
    alpha = np.float32(np.float32(np.max(np.abs(v))) + np.float32(1e-12))
    scale = np.float32(alpha / np.float32(7.0))
    q = np.round(np.clip(v, -alpha, alpha) / scale).astype(np.float32)
    return q, scale


def _fold_bn(gamma, beta, mean, var):
    gamma = np.asarray(gamma, np.float32)
    beta = np.asarray(beta, np.float32)
    mean = np.asarray(mean, np.float32)
    var = np.asarray(var, np.float32)
    inv = (gamma / np.sqrt(var + EPS)).astype(np.float32)
    b = (beta - mean * inv).astype(np.float32)
    return inv, b


# ------------------------------------------------------------------ bass IR
def _build():
    global _BUILT
    if _BUILT is not None:
        return _BUILT
    import concourse.bacc as bacc
    import concourse.tile as tile
    from concourse import mybir
    from contextlib import ExitStack

    f32 = mybir.dt.float32
    f8 = mybir.dt.float8e4
    AF = mybir.ActivationFunctionType
    OP = mybir.AluOpType
    DR = mybir.MatmulPerfMode.DoubleRow
    AX = mybir.AxisListType

    # inpa2 carries img1 AND the conv1/cot1 weights: cot1 matmuls start at
    # ~stream+6.2us but inpb1's receipt (chained behind inpa2) lands ~+7us,
    # which showed up as a ~0.4us stall at the first cot1 matmul.
    SA0, SA1, SA, SB1 = W0bOFF, X1OFF, WROFF + 9 * WG, X2OFF + 2 * IMB
    nc = bacc.Bacc("TRN2", target_bir_lowering=False, debug=False)
    inpa_d = nc.dram_tensor("inpa", [128, SA0], f8, kind="ExternalInput").ap()
    inpa1b_d = nc.dram_tensor("inpa1b", [128, SA1 - SA0], f8, kind="ExternalInput").ap()
    inpa2_d = nc.dram_tensor("inpa2", [128, SA - SA1], f8, kind="ExternalInput").ap()
    inpb1_d = nc.dram_tensor("inpb1", [128, SB1 - SA], f8, kind="ExternalInput").ap()
    inpb2_d = nc.dram_tensor("inpb2", [128, INPB - SB1], f8, kind="ExternalInput").ap()
    r_d = nc.dram_tensor("resid", [128, 2, BC, 2, 14, 28], f32, kind="ExternalInput").ap()
    y_d = nc.dram_tensor("y", [2, 128, BC, 2, 14, 28], f32, kind="ExternalOutput").ap()
    am_d = nc.dram_tensor("amax", [128, 4], f32, kind="ExternalOutput").ap()

    with tile.TileContext(nc) as tc, ExitStack() as ctx:
        const = ctx.enter_context(tc.tile_pool(name="const", bufs=1))
        psum = ctx.enter_context(tc.tile_pool(name="psum", bufs=8, space="PSUM"))
        ep1 = ctx.enter_context(tc.tile_pool(name="ep1", bufs=4))
        ep2 = ctx.enter_context(tc.tile_pool(name="ep2", bufs=4))
        yp = ctx.enter_context(tc.tile_pool(name="yp", bufs=3))

        inp_sb = const.tile([128, INPB], f8, tag="inp")
        x2_sb = const.tile([128, BC, 2, IMS], f8, tag="x2")
        rs_sb = const.tile([128, 2, BC, 2, 14, 28], f32, tag="rs")
        am_sb = const.tile([128, 4], f32, tag="am")

        vecv = inp_sb[:, VOFF:VOFF + 40].bitcast(f32)      # [128, 10] f32

        def w_ap(g):   # [128, 2, 128] tap g in (ci,cot)-major order
            off = (g * WG if g < 3 else
                   W0bOFF + (g - 3) * WG if g < 9 else
                   WROFF + (g - 9) * WG)
            return inp_sb[:, off:off + WG].rearrange("p (r m) -> p r m", r=2)

        def x1_ap(b):  # [128, 2, IMS] image b
            off = (X0OFF if b == 0 else X1OFF if b == 1 else
                   X2OFF + (b - 2) * IMB)
            return inp_sb[:, off:off + IMB].rearrange("p (r s) -> p r s", r=2)

        # HAM pre-warm: junk matmuls on zeroed SBUF during the input-DMA
        # window so the PE clock gate is already at 2.4GHz (warm) when the
        # first real matmul issues (~3.4us of sustained activity required).
        wj = const.tile([128, 256], f8, tag="wj")
        nc.vector.memset(wj[:], 0.0)
        jl = wj[:].rearrange("p (r m) -> p r m", r=2)
        # N=128 junks: measured to warm the HAM in ~2.9us of activity vs
        # ~4us for N=64 ones; 28 of them (~3.0us cold) end right at the
        # gating chunk's DMA receipt (~issue+3.2us) without overshooting
        jp = psum.tile([128, NT], f32, tag="pt", name="jp")
        for _ in range(28):
            nc.tensor.matmul(jp[:, 0:128], jl, jl, start=True, stop=True,
                             perf_mode=DR)

        from concourse.tile_rust import add_dep_helper
        dma_a = nc.sync.dma_start(inp_sb[:, 0:SA0], inpa_d)
        dma_a1b = nc.sync.dma_start(inp_sb[:, SA0:SA1], inpa1b_d)
        dma_a2 = nc.sync.dma_start(inp_sb[:, SA1:SA], inpa2_d)
        dma_b1 = nc.sync.dma_start(inp_sb[:, SA:SB1], inpb1_d)
        dma_b2 = nc.sync.dma_start(inp_sb[:, SB1:INPB], inpb2_d)
        dma_r = nc.sync.dma_start(rs_sb[:], r_d)
        # head chunks: ordering-only deps (same engine + queue -> FIFO drain
        # right behind the minimal gating chunk); tail chunks: receipt-chained
        # so the aggregate HBM load stays staggered across the 8 cores
        for a, b in ((dma_a1b, dma_a), (dma_a2, dma_a1b)):
            add_dep_helper(a.ins, b.ins, sync=False,
                           reason="issue head DMA chunks in first-use order")
        for a, b in ((dma_b1, dma_a2), (dma_b2, dma_b1), (dma_r, dma_b2)):
            add_dep_helper(a.ins, b.ins, sync=True,
                           reason="stage input DMAs by first-use order")
        nc.gpsimd.memset(x2_sb[:], 0.0)

        def vcol(i):
            return vecv[:, i : i + 1]

        def valid(apnt):   # [128,406] -> [128,14,28] dropping 1 garbage col/row
            return apnt.rearrange("p (h w) -> p h w", w=29)[:, :, :28]

        for ci, src in ((0, None), (1, x2_sb)):
            # conv2 tapers to 1-image phases so the final epilogue tail is short
            groups = ([(b0, 2) for b0 in range(0, BC, 2)] if ci == 0 else
                      [(0, 2), (2, 2), (4, 2), (6, 1), (7, 1)])
            for b0, gsz in groups:
                for cot in range(2):
                    pts = {}
                    # -- 9 taps x (gsz images x 2 row-halves) per weight --
                    # The very first phase is image-major (weights reloaded
                    # per image; LDWEIGHTS stays hidden) so the stream starts
                    # on img0's DMA receipt without waiting for img1's.
                    if ci == 0 and b0 == 0 and cot == 0:
                        order = [(k, bb, hb) for bb in range(gsz)
                                 for k in range(9) for hb in range(2)]
                    else:
                        order = [(k, bb, hb) for k in range(9)
                                 for bb in range(gsz) for hb in range(2)]
                    for k, bb, hb in order:
                        off = (k // 3) * 29 + (k % 3)
                        lhsT = w_ap((ci * 2 + cot) * 9 + k)
                        b = b0 + bb
                        if k == 0:
                            pts[(bb, hb)] = psum.tile(
                                [128, NT], f32, tag="pt", name="pt")
                        s = hb * NT + off
                        rhs = (x1_ap(b) if ci == 0 else
                               src[:, b, :, :])[:, :, s : s + NT]
                        nc.tensor.matmul(
                            pts[(bb, hb)][:], lhsT, rhs,
                            start=(k == 0), stop=(k == 8), perf_mode=DR)
                    # ---- epilogues for this phase's psum tiles ----
                    for bb in range(gsz):
                        b = b0 + bb
                        if ci == 1:
                            yb = yp.tile([128, 2, 14, 28], f32, tag="yb", name="yb")
                        for hb in range(2):
                            pt3 = valid(pts[(bb, hb)][:])
                            if ci == 0:
                                # t=P*a1+b1p ; clip +-7 ; +-MAGIC rne -> fp8
                                t1 = ep1.tile([128, 14, 28], f32, tag="t1", name="t1")
                                nc.scalar.activation(
                                    t1[:], pt3, AF.Identity,
                                    bias=vcol(2 + cot), scale=vcol(0 + cot))
                                t2 = ep1.tile([128, 14, 28], f32, tag="t2", name="t2")
                                nc.vector.tensor_scalar(
                                    t2[:], t1[:], 7.0, -7.0, op0=OP.min, op1=OP.max)
                                if bb == 0 and hb == 0 and b0 in (0, 2):
                                    # any tile hitting exactly 7.0 proves
                                    # alpha2 == 1.0 globally (clip bound)
                                    idx = (b0 // 2) * 2 + cot
                                    nc.vector.tensor_reduce(
                                        am_sb[:, idx : idx + 1], t2[:], op=OP.max,
                                        axis=AX.XY, apply_absolute_value=True)
                                t3 = ep1.tile([128, 14, 28], f32, tag="t3", name="t3")
                                nc.scalar.activation(
                                    t3[:], t2[:], AF.Copy, bias=float(MAGIC), scale=1.0)
                                dst = valid(
                                    x2_sb[:, b, cot, hb * NT + 30 : hb * NT + 30 + NT])
                                nc.vector.tensor_scalar(
                                    dst, t3[:], -float(MAGIC), None, op0=OP.add)
                            else:
                                # y = clip(P2*c2 + (x*inv2 + b2), +-1);
                                # the residual affine is precomputed on host
                                u3 = ep2.tile([128, 14, 28], f32, tag="u3", name="u3")
                                nc.vector.scalar_tensor_tensor(
                                    u3[:], pt3, vcol(4 + cot), rs_sb[:, cot, b, hb],
                                    op0=OP.mult, op1=OP.add)
                                nc.vector.tensor_scalar(
                                    yb[:, hb], u3[:], 1.0, -1.0,
                                    op0=OP.min, op1=OP.max)
                                if gsz == 1:
                                    nc.sync.dma_start(y_d[cot, :, b, hb],
                                                      yb[:, hb])
                        if ci == 1 and gsz > 1:
                            nc.sync.dma_start(y_d[cot, :, b], yb[:])
            if ci == 0:
                nc.sync.dma_start(am_d, am_sb[:])

    nc.compile()
    _dedupe_ldweights(nc)
    _BUILT = (nc,)
    return _BUILT


# ------------------------------------------------------------- input packing
def _prep(x, w1, w2, inv1, b1, inv2, b2):
    xi, s_x = _quant_int(x)
    w1i, s_w1 = _quant_int(w1)
    w2i, s_w2 = _quant_int(w2)

    xi8 = xi.astype(F8NP)
    tmp = np.zeros((NCORES, BC, 2, 128, 30, 29), F8NP)
    tmp[:, :, :, :, 1:29, 1:29] = xi8.reshape(NCORES, BC, 2, 128, 28, 28)
    x1_all = np.zeros((NCORES, 128, BC, 2, IMS), F8NP)
    x1_all[..., :870] = tmp.transpose(0, 3, 1, 2, 4, 5).reshape(
        NCORES, 128, BC, 2, 870)

    def wpack(wi):
        # w[cot*128+m, r*128+p, kh, kw] -> [p, (cot,k), r, m]
        v = wi.reshape(2, 128, 2, 128, 9)          # cot, m, r, p, k
        v = v.transpose(3, 0, 4, 2, 1)             # p, cot, k, r, m
        return v.reshape(128, 18, 2, 128).astype(F8NP)

    w_all = np.concatenate([wpack(w1i), wpack(w2i)], axis=1).reshape(128, WB)

    s2 = np.float32(np.float32(1.0) / np.float32(7.0))
    a1 = (np.float32(7.0) * s_x * s_w1 * inv1).astype(np.float32)
    b1p = (np.float32(7.0) * b1).astype(np.float32)
    c2 = (s2 * s_w2 * inv2).astype(np.float32)
    cols = [a1[:128], a1[128:], b1p[:128], b1p[128:], c2[:128], c2[128:],
            inv2[:128], inv2[128:], b2[:128], b2[128:]]
    vec8 = np.zeros((128, VB), F8NP)
    vec8[:, :40] = np.ascontiguousarray(
        np.stack(cols, axis=1).astype(np.float32)).view(F8NP)

    # residual affine x*inv2 + b2, precomputed -> [cores, 128(m), 2(cot), BC, ...]
    rs2 = (x * inv2[None, :, None, None] + b2[None, :, None, None]).astype(np.float32)
    resid = rs2.reshape(NCORES, BC, 2, 128, 2, 14, 28).transpose(0, 3, 2, 1, 4, 5, 6)
    resid = np.ascontiguousarray(resid)

    in_maps = []
    for i in range(NCORES):
        x1i = x1_all[i].reshape(128, X1B)
        inpa = np.concatenate([w_all[:, :3 * WG], vec8, x1i[:, :IMB]], axis=1)
        inpa1b = w_all[:, 3 * WG:9 * WG]
        inpa2 = np.concatenate([x1i[:, IMB:2 * IMB],
                                w_all[:, W0B:W0B + 9 * WG]], axis=1)
        inpb1 = np.concatenate([w_all[:, W0B + 9 * WG:],
                                x1i[:, 2 * IMB:4 * IMB]], axis=1)
        inpb2 = x1i[:, 4 * IMB:]
        in_maps.append({"inpa": np.ascontiguousarray(inpa),
                        "inpa1b": np.ascontiguousarray(inpa1b),
                        "inpa2": np.ascontiguousarray(inpa2),
                        "inpb1": np.ascontiguousarray(inpb1),
                        "inpb2": np.ascontiguousarray(inpb2),
                        "resid": resid[i]})
    return in_maps, (xi, w1i, w2i, s_x, s_w1, s_w2, s2)


# ------------------------------------------------------- exact numpy fallback
def _conv3x3_int(xint, wint):
    Bn, Cn, Hn, Wn = xint.shape
    xp = np.zeros((Bn, Cn, Hn + 2, Wn + 2), np.float64)
    xp[:, :, 1:-1, 1:-1] = xint
    out = np.zeros((Bn, wint.shape[0], Hn, Wn), np.float64)
    w64 = wint.astype(np.float64)
    for kh in range(3):
        for kw in range(3):
            out += np.einsum("bchw,oc->bohw", xp[:, :, kh:kh + Hn, kw:kw + Wn],
                             w64[:, :, kh, kw], optimize=True)
    return out.astype(np.float32)


def _numpy_path(x, q, inv1, b1, inv2, b2):
    """Exact replica handling arbitrary alpha2 (never expected to run)."""
    xi, w1i, w2i, s_x, s_w1, s_w2, _ = q
    P1 = _conv3x3_int(xi, w1i)
    h = (P1 * (s_x * s_w1 * inv1)[None, :, None, None]).astype(np.float32)
    h = (h + b1[None, :, None, None]).astype(np.float32)
    h = np.clip(h, np.float32(-1.0), np.float32(1.0))
    alpha2 = np.float32(np.abs(h).max())
    s2 = np.float32(alpha2 / np.float32(7.0))
    x2 = np.round(np.clip(h, -alpha2, alpha2) / s2).astype(np.float32)
    P2 = _conv3x3_int(x2, w2i)
    u = (P2 * (s2 * s_w2 * inv2)[None, :, None, None]).astype(np.float32)
    u = (u + (x * inv2[None, :, None, None] + b2[None, :, None, None])).astype(np.float32)
    return np.clip(u, np.float32(-1.0), np.float32(1.0))


# ------------------------------------------------------------------- kernel
def _dedupe_ldweights(nc):
    """Drop InstLdweights that reload the stationary operand already in the
    PE array (consecutive matmuls here reuse one weight 8x).  Safe because
    Ldweights carry no semaphore updates; ones carrying waits are kept."""
    for f in nc.m.functions:
        for blk in f.blocks:
            il = blk.instructions
            keep, last_sig, removed = [], None, 0
            for ins in il:
                tn = type(ins).__name__
                if tn == "InstLdweights":
                    sig = (str(ins.ins), str(ins.perf_mode),
                           str(ins.tile_position), str(ins.is_transpose))
                    plain = ("wait:" not in str(ins)
                             and "update:" not in str(ins))
                    if sig == last_sig and plain:
                        removed += 1
                        continue
                    last_sig = sig
                elif tn in ("InstMatmult", "InstEventSemaphore", "InstDrain"):
                    pass                     # none of these clobber loaded weights
                elif str(getattr(ins, "engine", "")).endswith("PE"):
                    last_sig = None          # conservative reset on other PE ops
                keep.append(ins)
            if removed:
                il[:] = keep


def _run(in_maps, trace=False, tmpdir=None):
    from concourse.bass_utils import run_bass_kernel_spmd
    (nc,) = _build()
    return run_bass_kernel_spmd(nc, in_maps, list(range(NCORES)), trace=trace,
                                tmpdir=tmpdir)


def kernel(x, w1, bn1_gamma, bn1_beta, bn1_mean, bn1_var,
           w2, bn2_gamma, bn2_beta, bn2_mean, bn2_var):
    x = np.asarray(x, np.float32)
    w1 = np.asarray(w1, np.float32)
    w2 = np.asarray(w2, np.float32)
    inv1, b1 = _fold_bn(bn1_gamma, bn1_beta, bn1_mean, bn1_var)
    inv2, b2 = _fold_bn(bn2_gamma, bn2_beta, bn2_mean, bn2_var)

    in_maps, q = _prep(x, w1, w2, inv1, b1, inv2, b2)
    res = _run(in_maps)

    amax = np.max([r["amax"] for r in res.results])
    if not np.float32(amax) == np.float32(7.0):
        return _numpy_path(x, q, inv1, b1, inv2, b2)

    ys = np.stack([r["y"] for r in res.results])      # [cores, 2, 128, BC, 2,14,28]
    ys = ys.reshape(NCORES, 2, 128, BC, 784)
    return ys.transpose(0, 3, 1, 2, 4).reshape(B, C, H, W).copy()

